# revision 13
# baseline (speedup 1.0000x reference)
"""Trainium2 Bass kernel: ContinuousConvolution (KNN gather + linear kernel-MLP).

Math (per batch b, point n):
  idx      = 16 nearest neighbors of n by squared distance (self first)
  g_k      = [pf[idx_k], coords[idx_k] - coords[n]]            (67 ch)
  y_pool   = max_k pf[idx_k]                                   (64)
  h_k      = W3(W2(W1 g_k + b1) + b2) + b3 = W g_k + c  (no activations!)
  out_sum  = sum_k h_k      = W (sum_k g_k)       + 16 c
  y_aggr   = sum_k w_k h_k  = W (sum_k w_k g_k)   + (sum w) c + aggr_b
  out      = [out_sum | y_pool | y_aggr]                       (192)

Distribution: 8 cores = 2 batches x 4 row-shards of 2048 points.
Each core: PE computes score rows s[i,j] = 2<c_i,c_j> - |c_j|^2 (monotone in
-d2), DVE hardware top-k (max8/max_index/match_replace x2) -> 16 indices,
indirect-DMA gather of neighbor rows from HBM, GPSIMD/ACT reductions, PE for
the fused 67->64 linear map, transposes to keep point-major layout.
"""

import numpy as np
from contextlib import ExitStack

import concourse.bass as bass
import concourse.bacc as bacc
import concourse.mybir as mybir
import concourse.tile as tile
from concourse.bass import IndirectOffsetOnAxis
from concourse.bass_utils import run_bass_kernel_spmd
from concourse.masks import make_identity

B, N, C_IN, CC, K = 2, 8192, 64, 3, 16
C_CAT = C_IN + CC            # 67
HID1, HID2, C_OUT = 32, 64, 64
OUT_C = 3 * C_OUT            # 192
NCORES = 8
SHARDS_PER_B = NCORES // B   # 4
R = N // SHARDS_PER_B        # 2048 rows per core
P = 128                      # partitions / rows per block
MM_F = 512                   # matmul free-dim (one PSUM bank of fp32)

f32 = mybir.dt.float32
u32 = mybir.dt.uint32
NEG_BIG = -1.0e30


def build_program(n_tbl: int = N, rows: int = R, dump: bool = False):
    """Build + compile the per-core program (identical across cores)."""
    nblk = rows // P
    ncol = n_tbl // MM_F
    nc = bacc.Bacc(
        "TRN2",
        target_bir_lowering=False,
        debug=False,
        enable_asserts=False,
        num_devices=NCORES,
    )

    feats = nc.dram_tensor("feats", [n_tbl, C_CAT], f32, kind="ExternalInput").ap()
    coordsT = nc.dram_tensor("coordsT", [CC, n_tbl], f32, kind="ExternalInput").ap()
    rowsT = nc.dram_tensor("rowsT", [CC, rows], f32, kind="ExternalInput").ap()
    rows_pm = nc.dram_tensor("rows_pm", [rows, CC], f32, kind="ExternalInput").ap()
    wt_d = nc.dram_tensor("wt", [C_CAT, C_OUT], f32, kind="ExternalInput").ap()
    csum_d = nc.dram_tensor("csum", [P, C_OUT], f32, kind="ExternalInput").ap()
    caggr_d = nc.dram_tensor("caggr", [P, C_OUT], f32, kind="ExternalInput").ap()
    aggrw_d = nc.dram_tensor("aggrw", [P, K], f32, kind="ExternalInput").ap()
    wsumn_d = nc.dram_tensor("wsumn", [P, 1], f32, kind="ExternalInput").ap()
    out_d = nc.dram_tensor("out", [rows, OUT_C], f32, kind="ExternalOutput").ap()
    if dump:
        dbg_s = nc.dram_tensor("dbg_s", [P, n_tbl], f32, kind="ExternalOutput").ap()
        dbg_i = nc.dram_tensor("dbg_i", [rows, 2 * 8], u32, kind="ExternalOutput").ap()
        dbg_v = nc.dram_tensor("dbg_v", [rows, 2 * 8], f32, kind="ExternalOutput").ap()
        dbg_g = nc.dram_tensor("dbg_g", [P, K * C_CAT], f32, kind="ExternalOutput").ap()

    with tile.TileContext(nc) as tc, ExitStack() as ctx:
        const = ctx.enter_context(tc.tile_pool(name="const", bufs=1))
        spool = ctx.enter_context(tc.tile_pool(name="score", bufs=2))
        gpool = ctx.enter_context(tc.tile_pool(name="gath", bufs=2))
        redp = ctx.enter_context(tc.tile_pool(name="red", bufs=2))
        smallp = ctx.enter_context(tc.tile_pool(name="small", bufs=3))
        opool = ctx.enter_context(tc.tile_pool(name="outp", bufs=2))
        psA = ctx.enter_context(tc.tile_pool(name="psA", bufs=3, space="PSUM"))
        psB = ctx.enter_context(tc.tile_pool(name="psB", bufs=2, space="PSUM"))
        psC = ctx.enter_context(tc.tile_pool(name="psC", bufs=1, space="PSUM"))

        # ---- one-time setup ----
        # score s[p, j] = 2<c_p, c_j> - |c_j|^2 via two accumulating matmuls:
        #   mm1: lhsT = 2*c_rows [3, P],  rhs = coordsT   [3, f]
        #   mm2: lhsT = -1       [3, P],  rhs = coordsT^2 [3, f]  (accumulate)
        rhs_c = const.tile([CC, n_tbl], f32)
        rhs_sq = const.tile([CC, n_tbl], f32)
        lhs_2c = const.tile([CC, rows], f32)
        negones = const.tile([CC, P], f32)
        nc.sync.dma_start(out=rhs_c[:], in_=coordsT[:, :])
        nc.vector.tensor_tensor(
            out=rhs_sq[:], in0=rhs_c[:], in1=rhs_c[:], op=mybir.AluOpType.mult,
        )
        nc.sync.dma_start(out=lhs_2c[:], in_=rowsT[:, :])
        nc.vector.tensor_scalar_mul(lhs_2c[:], lhs_2c[:], 2.0)
        nc.vector.memset(negones[:], -1.0)

        wt_sb = const.tile([C_CAT, C_OUT], f32)
        nc.sync.dma_start(out=wt_sb[:], in_=wt_d[:, :])
        csum_sb = const.tile([P, C_OUT], f32)
        nc.sync.dma_start(out=csum_sb[:], in_=csum_d[:, :])
        caggr_sb = const.tile([P, C_OUT], f32)
        nc.sync.dma_start(out=caggr_sb[:], in_=caggr_d[:, :])
        aggrw_sb = const.tile([P, K], f32)
        nc.sync.dma_start(out=aggrw_sb[:], in_=aggrw_d[:, :])
        wsumn_sb = const.tile([P, 1], f32)
        nc.sync.dma_start(out=wsumn_sb[:], in_=wsumn_d[:, :])
        ident = const.tile([P, P], f32)
        make_identity(nc, ident[:])
        rows_sb = const.tile([P, nblk * CC], f32)
        for nb in range(nblk):
            nc.sync.dma_start(
                out=rows_sb[:, nb * CC:(nb + 1) * CC],
                in_=rows_pm[nb * P:(nb + 1) * P, :],
            )

        # ---- per row-block ----
        for nb in range(nblk):
            # phase A: scores s[p, j] = 2<c_p, c_j> - |c_j|^2   (PE -> ACT)
            s = spool.tile([P, n_tbl], f32, tag="s")
            for ch in range(ncol):
                ps = psA.tile([P, MM_F], f32, tag="ps")
                nc.tensor.matmul(
                    ps[:],
                    lhsT=lhs_2c[:, nb * P:(nb + 1) * P],
                    rhs=rhs_c[:, ch * MM_F:(ch + 1) * MM_F],
                    start=True, stop=False,
                )
                nc.tensor.matmul(
                    ps[:],
                    lhsT=negones[:],
                    rhs=rhs_sq[:, ch * MM_F:(ch + 1) * MM_F],
                    start=False, stop=True,
                )
                nc.scalar.copy(out=s[:, ch * MM_F:(ch + 1) * MM_F], in_=ps[:])

            if dump and nb == 0:
                nc.sync.dma_start(out=dbg_s[:, :], in_=s[:])

            # phase B: hardware top-16 (DVE)
            v16 = smallp.tile([P, 2 * 8], f32, tag="v16")
            i16 = smallp.tile([P, 2 * 8], u32, tag="i16")
            nc.vector.max(out=v16[:, 0:8], in_=s[:])
            nc.vector.max_index(out=i16[:, 0:8], in_max=v16[:, 0:8], in_values=s[:])
            nc.vector.match_replace(
                out=s[:], in_to_replace=v16[:, 0:8], in_values=s[:],
                imm_value=NEG_BIG,
            )
            nc.vector.max(out=v16[:, 8:16], in_=s[:])
            nc.vector.max_index(out=i16[:, 8:16], in_max=v16[:, 8:16], in_values=s[:])

            if dump:
                nc.sync.dma_start(out=dbg_i[nb * P:(nb + 1) * P, :], in_=i16[:])
                nc.sync.dma_start(out=dbg_v[nb * P:(nb + 1) * P, :], in_=v16[:])

            # phase C: gather 16 neighbor rows per point from HBM.
            # HW indirect DMA supports ONE offset per partition (one
            # descriptor per partition), so issue one DMA per neighbor slot.
            g = gpool.tile([P, K * C_CAT], f32, tag="g")
            for k in range(K):
                nc.gpsimd.indirect_dma_start(
                    out=g[:, k * C_CAT:(k + 1) * C_CAT],
                    out_offset=None,
                    in_=feats[:, :],
                    in_offset=IndirectOffsetOnAxis(ap=i16[:, k:k + 1], axis=0),
                )

            if dump and nb == 0:
                nc.sync.dma_start(out=dbg_g[:, :], in_=g[:])

            out_t = opool.tile([P, OUT_C], f32, tag="out_t")
            t01 = smallp.tile([P, 2 * C_CAT], f32, tag="t01")

            # strided views of the gathered tile: [P, k, c] and [P, c, k]
            g3 = g[:].rearrange("p (k c) -> p k c", k=K)
            gT = g[:].rearrange("p (k c) -> p c k", k=K)

            # phase D1: y_pool = max_k pf[idx_k]
            nc.vector.tensor_reduce(
                out=out_t[:, C_OUT:2 * C_OUT], in_=gT[:, 0:C_IN, :],
                axis=mybir.AxisListType.X, op=mybir.AluOpType.max,
            )
            # phase D2: T0 = sum_k g_k
            nc.vector.tensor_reduce(
                out=t01[:, 0:C_CAT], in_=gT,
                axis=mybir.AxisListType.X, op=mybir.AluOpType.add,
            )
            # phase D3: T1 = sum_k w_k g_k
            gw = redp.tile([P, K * C_CAT], f32, tag="gw")
            nc.vector.tensor_tensor(
                out=gw[:].rearrange("p (k c) -> p k c", k=K),
                in0=g3,
                in1=aggrw_sb[:].unsqueeze(2).to_broadcast([P, K, C_CAT]),
                op=mybir.AluOpType.mult,
            )
            nc.vector.tensor_reduce(
                out=t01[:, C_CAT:2 * C_CAT],
                in_=gw[:].rearrange("p (k c) -> p c k", k=K),
                axis=mybir.AxisListType.X, op=mybir.AluOpType.add,
            )

            # phase D4: relative-coord corrections
            rb = rows_sb[:, nb * CC:(nb + 1) * CC]
            nc.vector.scalar_tensor_tensor(
                out=t01[:, C_IN:C_CAT],
                in0=rb, scalar=-float(K), in1=t01[:, C_IN:C_CAT],
                op0=mybir.AluOpType.mult, op1=mybir.AluOpType.add,
            )
            nc.vector.scalar_tensor_tensor(
                out=t01[:, C_CAT + C_IN:2 * C_CAT],
                in0=rb, scalar=wsumn_sb[:, 0:1], in1=t01[:, C_CAT + C_IN:2 * C_CAT],
                op0=mybir.AluOpType.mult, op1=mybir.AluOpType.add,
            )

            # phase E: fused linear map, point-major via PE transpose
            t01t = smallp.tile([C_CAT, 2 * P], f32, tag="t01t")
            for half in range(2):
                pt = psB.tile([C_CAT, P], f32, tag="pt")
                nc.tensor.transpose(
                    out=pt[:],
                    in_=t01[:, half * C_CAT:(half + 1) * C_CAT],
                    identity=ident[:],
                )
                nc.scalar.copy(out=t01t[:, half * P:(half + 1) * P], in_=pt[:])
            po = psC.tile([P, C_OUT], f32, tag="po")
            nc.tensor.matmul(
                po[:], lhsT=t01t[:, 0:P], rhs=wt_sb[:], start=True, stop=True,
            )
            nc.vector.tensor_tensor(
                out=out_t[:, 0:C_OUT], in0=po[:], in1=csum_sb[:],
                op=mybir.AluOpType.add,
            )
            po2 = psC.tile([P, C_OUT], f32, tag="po2")
            nc.tensor.matmul(
                po2[:], lhsT=t01t[:, P:2 * P], rhs=wt_sb[:], start=True, stop=True,
            )
            nc.vector.tensor_tensor(
                out=out_t[:, 2 * C_OUT:3 * C_OUT], in0=po2[:], in1=caggr_sb[:],
                op=mybir.AluOpType.add,
            )

            nc.sync.dma_start(
                out=out_d[nb * P:(nb + 1) * P, :], in_=out_t[:],
            )

    nc.compile()
    return nc


_PROG_CACHE: dict = {}


def _get_program(n_tbl=N, rows=R):
    key = (n_tbl, rows)
    if key not in _PROG_CACHE:
        _PROG_CACHE[key] = build_program(n_tbl, rows)
    return _PROG_CACHE[key]


def make_in_maps(point_features, coords, w1, b1, w2, b2, w3, b3, aggr_w, aggr_b,
                 n_tbl=N, rows=R, ncores=NCORES):
    pf = np.asarray(point_features, np.float32)
    co = np.asarray(coords, np.float32)
    w1 = np.asarray(w1, np.float32); b1 = np.asarray(b1, np.float32)
    w2 = np.asarray(w2, np.float32); b2 = np.asarray(b2, np.float32)
    w3 = np.asarray(w3, np.float32); b3 = np.asarray(b3, np.float32)
    aggr_w = np.asarray(aggr_w, np.float32)
    aggr_b = np.asarray(aggr_b, np.float32)

    nb = pf.shape[0]
    shards = ncores // nb

    W = (w3 @ w2 @ w1).astype(np.float32)            # [64, 67]
    c = (w3 @ (w2 @ b1 + b2) + b3).astype(np.float32)  # [64]
    wsum = np.float32(aggr_w.sum())
    wt = np.ascontiguousarray(W.T)                   # [67, 64]
    csum = np.tile(np.float32(K) * c, (P, 1))
    caggr = np.tile(wsum * c + aggr_b.astype(np.float32), (P, 1))
    aggrw_bc = np.tile(aggr_w, (P, 1))
    wsumn = np.full((P, 1), -wsum, np.float32)

    in_maps = []
    for core in range(ncores):
        b = core // shards
        r0 = (core % shards) * rows
        feats_b = np.ascontiguousarray(
            np.concatenate([pf[b], co[b]], axis=-1), np.float32)
        in_maps.append({
            "feats": feats_b,
            "coordsT": np.ascontiguousarray(co[b].T),
            "rowsT": np.ascontiguousarray(co[b, r0:r0 + rows].T),
            "rows_pm": np.ascontiguousarray(co[b, r0:r0 + rows]),
            "wt": wt,
            "csum": np.ascontiguousarray(csum),
            "caggr": np.ascontiguousarray(caggr),
            "aggrw": np.ascontiguousarray(aggrw_bc),
            "wsumn": wsumn,
        })
    return in_maps


def kernel(point_features, coords, w1, b1, w2, b2, w3, b3, aggr_w, aggr_b,
           **_unused):
    pf = np.asarray(point_features, np.float32)
    nb_, n_, _ = pf.shape
    nc = _get_program(N, R)
    in_maps = make_in_maps(point_features, coords, w1, b1, w2, b2, w3, b3,
                           aggr_w, aggr_b)
    res = run_bass_kernel_spmd(nc, in_maps, list(range(NCORES)))
    out = np.zeros((B, N, OUT_C), np.float32)
    for core in range(NCORES):
        b = core // SHARDS_PER_B
        r0 = (core % SHARDS_PER_B) * R
        out[b, r0:r0 + R] = res.results[core]["out"]
    return out
